# revision 1
# baseline (speedup 1.0000x reference)
"""MegablockMoE kernel for 8 Trainium2 NeuronCores.

Expert-parallel (one expert per core); router/dispatch on host.

v5: minimum PE instruction count — every matmul at N=512 (max moving free).
  mm1 (c_tile=512): hT[f, c] = gelu(w1.T @ xg); stationary w1[o,f] blocks,
      moving xg (N=512); 1024 MMs total.
  mm2 (per 128-token block): y[c, d] = hT.T @ w2; stationary hT[f, cb]
      (reused for both d-halves), moving w2 rows (N=512), accumulated over
      all 32 f-tiles into 2 whole PSUM banks; 1024 MMs total.
  hT is single-buffered: mm2(t) directly follows mm1(t) on the PE, and its
  f-sequential accumulation only touches hT[f=31] ~13us into the phase, so
  the trailing gelu eviction latency is naturally hidden.
  DMA: weights on the SP HWDGE queue, tokens/outputs on the ACT queue.
"""

import numpy as np
import ml_dtypes

import concourse.mybir as mybir
import concourse.tile as tile
from concourse import bacc
from concourse.bass_utils import run_bass_kernel_spmd

B, S, D = 4, 2048, 1024
E, K, DFF = 8, 2, 4096
T = B * S
C = K * T // E  # 2048 expert capacity
BF16 = ml_dtypes.bfloat16
N_CORES = 8

KO1, KO2 = D // 128, DFF // 128  # 8, 32
W1_CH = 16           # f-chunks of w1 (separate tiles -> fine-grained DMA deps)
W1_F = DFF // W1_CH  # 256
W2_CH = 8            # f-chunks of w2
W2_O = KO2 // W2_CH  # 4

_NC = None


def _build_nc(c_tile=512, ps1_bufs=3, ps2_bufs=4, xg_bufs=2, y_bufs=3,
              n_iters=1, debug=True):
    nc = bacc.Bacc(None, target_bir_lowering=False, debug=debug)
    xgT = nc.dram_tensor("xgT", [D, C], mybir.dt.bfloat16, kind="ExternalInput")
    w1 = nc.dram_tensor("w1", [D, DFF], mybir.dt.bfloat16, kind="ExternalInput")
    w2 = nc.dram_tensor("w2", [DFF, D], mybir.dt.bfloat16, kind="ExternalInput")
    y = nc.dram_tensor("y", [C, D], mybir.dt.float32, kind="ExternalOutput")

    xgT_v = xgT.rearrange("(o p) c -> p o c", p=128)   # [128, 8, C]
    w1_v = w1.rearrange("(o p) f -> p o f", p=128)     # [128, 8, DFF]
    w2_v = w2.rearrange("(f p) d -> p f d", p=128)     # [128, 32, D]
    n_ct = C // c_tile                                 # 4
    CB = c_tile // 128                                 # 4 c-blocks per c_tile

    with tile.TileContext(nc) as tc:
        with (
            tc.tile_pool(name="wpool", bufs=1) as wpool,
            tc.tile_pool(name="xpool", bufs=xg_bufs) as xpool,
            tc.tile_pool(name="hpool", bufs=1) as hpool,
            tc.tile_pool(name="ypool", bufs=y_bufs) as ypool,
            tc.tile_pool(name="ps1", bufs=ps1_bufs, space="PSUM") as ps1,
            tc.tile_pool(name="ps2", bufs=ps2_bufs, space="PSUM") as ps2,
        ):
            xg_tiles = {}

            def xg_dma(t):
                xg_tiles[t] = xpool.tile(
                    [128, KO1, c_tile], mybir.dt.bfloat16, tag="xg",
                    name=f"xg{t}",
                )
                cs = slice(t * c_tile, (t + 1) * c_tile)
                if t == 0:
                    # split per o-block: mm1(f=0,o) can start on chunk o
                    for o in range(KO1):
                        nc.scalar.dma_start(xg_tiles[t][:, o, :],
                                            xgT_v[:, o, cs])
                else:
                    nc.scalar.dma_start(xg_tiles[t][:], xgT_v[:, :, cs])

            if n_iters == 1:
                xg_dma(0)

            w1_tiles = []
            for ch in range(W1_CH):
                wt = wpool.tile([128, KO1, W1_F], mybir.dt.bfloat16,
                                tag=f"w1_{ch}", name=f"w1t{ch}")
                nc.sync.dma_start(wt[:], w1_v[:, :, ch * W1_F : (ch + 1) * W1_F])
                w1_tiles.append(wt)
            w2_tiles = []
            for ch in range(W2_CH):
                wt = wpool.tile([128, W2_O, D], mybir.dt.bfloat16,
                                tag=f"w2_{ch}", name=f"w2t{ch}")
                nc.sync.dma_start(wt[:], w2_v[:, ch * W2_O : (ch + 1) * W2_O, :])
                w2_tiles.append(wt)

            def w1_ap(o, f):
                ch, r = divmod(f, W1_F // 128)
                return w1_tiles[ch][:, o, r * 128 : (r + 1) * 128]

            def w2_ap(f, half):
                ch, r = divmod(f, W2_O)
                return w2_tiles[ch][:, r, half * 512 : (half + 1) * 512]

            def body(_=None):
                for t in range(n_ct):
                    if t == 0 and t not in xg_tiles:
                        xg_dma(0)
                    if t + 1 < n_ct:
                        xg_dma(t + 1)  # prefetch next c_tile during this one
                    xg_sb = xg_tiles[t]
                    hT_sb = hpool.tile([128, KO2, c_tile], mybir.dt.bfloat16,
                                       tag="hT", name=f"hT{t}")
                    # mm1: hT[f, c] = gelu(w1.T @ xg)
                    for f in range(KO2):
                        psum = ps1.tile([128, c_tile], mybir.dt.float32,
                                        tag="p1")
                        for o in range(KO1):
                            nc.tensor.matmul(
                                psum[:], w1_ap(o, f), xg_sb[:, o, :],
                                start=(o == 0), stop=(o == KO1 - 1),
                            )
                        nc.scalar.activation(
                            hT_sb[:, f, :], psum[:],
                            mybir.ActivationFunctionType.Gelu,
                        )
                    # mm2: y[c, d] = hT.T @ w2, one 128-token block at a time
                    for cb in range(CB):
                        pa = ps2.tile([128, 512], mybir.dt.float32, tag="p2",
                                      name=f"p2a{t}_{cb}")
                        pb = ps2.tile([128, 512], mybir.dt.float32, tag="p2",
                                      name=f"p2b{t}_{cb}")
                        for f in range(KO2):
                            lhsT = hT_sb[:, f, cb * 128 : (cb + 1) * 128]
                            nc.tensor.matmul(
                                pa[:], lhsT, w2_ap(f, 0),
                                start=(f == 0), stop=(f == KO2 - 1),
                            )
                            nc.tensor.matmul(
                                pb[:], lhsT, w2_ap(f, 1),
                                start=(f == 0), stop=(f == KO2 - 1),
                            )
                        y_sb = ypool.tile([128, D], mybir.dt.float32, tag="y",
                                          name=f"y{t}_{cb}")
                        # split evictions across DVE and ACT
                        nc.vector.tensor_copy(y_sb[:, 0:512], pa[:])
                        nc.scalar.copy(y_sb[:, 512:1024], pb[:])
                        r0 = t * c_tile + cb * 128
                        if t == n_ct - 1 and cb == CB - 1:
                            # last block: split the DMA across both queues to
                            # shorten the end-of-kernel drain
                            nc.scalar.dma_start(y[r0 : r0 + 128, 0:512],
                                                y_sb[:, 0:512])
                            nc.sync.dma_start(y[r0 : r0 + 128, 512:1024],
                                              y_sb[:, 512:1024])
                        else:
                            nc.scalar.dma_start(y[r0 : r0 + 128, :], y_sb[:])

            if n_iters == 1:
                body()
            else:
                with tc.For_i(0, n_iters, 1):
                    body()
    nc.compile()
    return nc


def _get_nc():
    global _NC
    if _NC is None:
        _NC = _build_nc()
    return _NC


def _route(x, wr):
    """Replicates the reference router exactly (fp32 numpy)."""
    xt = np.transpose(x, (1, 0, 2)).reshape(T, D)  # [T, D] fp32
    logits = xt.astype(np.float32) @ wr.astype(np.float32)  # [T, E]
    m = logits.max(axis=-1, keepdims=True)
    p = np.exp(logits - m, dtype=np.float32)
    p /= p.sum(axis=-1, keepdims=True)
    top1 = np.argmax(p, axis=-1)
    p_masked = p.copy()
    p_masked[np.arange(T), top1] = -np.inf
    top2 = np.argmax(p_masked, axis=-1)
    eidx = np.stack([top1, top2], axis=1)  # [T, K]
    ew = np.take_along_axis(p, eidx, axis=1).astype(np.float32)  # [T, K]

    flat_e = eidx.reshape(-1)
    order = np.argsort(flat_e, kind="stable")
    sorted_e = flat_e[order]
    hist = np.bincount(flat_e, minlength=E)
    starts = np.cumsum(hist) - hist
    pos = np.arange(T * K) - starts[sorted_e]
    keep = pos < C
    slot = np.where(keep, sorted_e * C + pos, E * C)
    token = order // K
    return xt, ew, order, keep, slot, token


def _make_in_maps(x, wr, w1, w2):
    xt, ew, order, keep, slot, token = _route(x, wr)
    slot_token = np.zeros(E * C, np.int64)
    slot_token[slot[keep]] = token[keep]
    xT_bf = np.ascontiguousarray(xt.T.astype(BF16))  # [D, T]
    in_maps = []
    for e in range(E):
        idx = slot_token[e * C : (e + 1) * C]
        in_maps.append(
            {
                "xgT": np.ascontiguousarray(xT_bf[:, idx]),
                "w1": np.ascontiguousarray(w1[e].astype(BF16)),
                "w2": np.ascontiguousarray(w2[e].astype(BF16)),
            }
        )
    return in_maps, (ew, order, keep, slot)


def kernel(x, wr, w1, w2):
    nc = _get_nc()
    in_maps, (ew, order, keep, slot) = _make_in_maps(x, wr, w1, w2)

    def _run_device():
        res = run_bass_kernel_spmd(nc, in_maps, core_ids=list(range(N_CORES)))
        # force materialization here so device errors surface inside the retry
        return [np.asarray(res.results[e]["y"], np.float32) for e in range(E)]

    try:
        Ys = _run_device()
    except Exception:
        # transient device errors (wedged core) usually clear on retry
        Ys = _run_device()

    # --- combine: weighted scatter back to tokens ---
    Y = np.empty((E * C, D), np.float32)
    for e in range(E):
        Y[e * C : (e + 1) * C] = Ys[e]

    inv = np.empty(T * K, np.int64)
    inv[order] = np.arange(T * K)
    slot_tk = slot[inv].reshape(T, K)
    keep_tk = keep[inv].reshape(T, K)

    out_flat = np.zeros((T, D), np.float32)
    for k in range(K):
        sl = np.clip(slot_tk[:, k], 0, E * C - 1)
        contrib = Y[sl] * ew[:, k : k + 1]
        contrib[~keep_tk[:, k]] = 0.0
        out_flat += contrib
    return np.ascontiguousarray(
        out_flat.reshape(S, B, D).transpose(1, 0, 2)
    ).astype(np.float32)


# ---------------------------------------------------------------------------
# Benchmark helper (used by test.py; not part of the grading contract).
# ---------------------------------------------------------------------------


def make_bench(in_maps):
    import jax
    from jax.experimental.shard_map import shard_map
    from jax.sharding import Mesh, PartitionSpec, NamedSharding
    from concourse.bass2jax import (
        _bass_exec_p,
        install_neuronx_cc_hook,
        partition_id_tensor,
    )

    nc = _NC if _NC is not None else _get_nc()
    install_neuronx_cc_hook()
    partition_name = nc.partition_id_tensor.name if nc.partition_id_tensor else None

    in_names, out_names, out_avals, zero_outs = [], [], [], []
    for alloc in nc.m.functions[0].allocations:
        if not isinstance(alloc, mybir.MemoryLocationSet):
            continue
        name = alloc.memorylocations[0].name
        if alloc.kind == "ExternalInput":
            if name != partition_name:
                in_names.append(name)
        elif alloc.kind == "ExternalOutput":
            shape = tuple(alloc.tensor_shape)
            dtype = mybir.dt.np(alloc.dtype)
            out_avals.append(jax.core.ShapedArray(shape, dtype))
            zero_outs.append(np.zeros(shape, dtype))
            out_names.append(name)
    n_params = len(in_names)
    all_in_names = list(in_names) + list(out_names)
    if partition_name is not None:
        all_in_names.append(partition_name)
    if nc.dbg_addr is not None:
        dbg_zero = np.zeros((1, 2), np.uint32)
        in_maps = [{**m, nc.dbg_addr.name: dbg_zero} for m in in_maps]

    def _body(*args):
        operands = list(args)
        if partition_name is not None:
            operands.append(partition_id_tensor())
        outs = _bass_exec_p.bind(
            *operands,
            out_avals=tuple(out_avals),
            in_names=tuple(all_in_names),
            out_names=tuple(out_names),
            lowering_input_output_aliases=(),
            sim_require_finite=True,
            sim_require_nnan=True,
            nc=nc,
        )
        return tuple(outs)

    devices = jax.devices()[:N_CORES]
    mesh = Mesh(np.asarray(devices), ("core",))
    n_outs = len(out_names)
    in_specs = (PartitionSpec("core"),) * (n_params + n_outs)
    out_specs = (PartitionSpec("core"),) * n_outs
    fn = jax.jit(
        shard_map(_body, mesh=mesh, in_specs=in_specs, out_specs=out_specs,
                  check_rep=False),
        keep_unused=True,
    )
    concat_in = [
        np.concatenate([np.asarray(in_maps[c][name]) for c in range(N_CORES)],
                       axis=0)
        for name in in_names
    ]
    concat_zeros = [
        np.zeros((N_CORES * z.shape[0], *z.shape[1:]), z.dtype)
        for z in zero_outs
    ]
    shard = NamedSharding(mesh, PartitionSpec("core"))
    args = [jax.device_put(a, shard) for a in concat_in + concat_zeros]
    return fn, args, out_names


def benchmark(in_maps, iters=20, warmup=3):
    import time
    import jax

    fn, args, out_names = make_bench(in_maps)
    for _ in range(warmup):
        out = fn(*args)
        jax.block_until_ready(out)
    times = []
    for _ in range(iters):
        t0 = time.perf_counter()
        out = fn(*args)
        jax.block_until_ready(out)
        times.append(time.perf_counter() - t0)
    return min(times), sorted(times)[len(times) // 2], out



# revision 3
# speedup vs baseline: 1.1487x; 1.1487x over previous
"""MegablockMoE kernel for 8 Trainium2 NeuronCores.

Expert-parallel (one expert per core); router/dispatch on host.

v7: bf16 mm1 everywhere + router-weight-tiered mm2.
  Within each expert, slots are host-permuted in ascending router-weight
  (ew) order.  The lowest-ew half of the slots (Q=8 of 16 blocks, which
  carries only ~15% of the ew^2-weighted output power) runs mm2 in
  fp8(e4m3) DoubleRow at 2x instruction throughput: stationary h8 =
  fp8(gelu(psum)) pairs over f, moving w2_8 = fp8(w2 * 2^12).  The
  remaining high-ew half runs mm2 in bf16 exactly as v5.  Measured
  rel-l2 error 1.5e-2 (gate 2e-2); the fp8 psum descale (2^-12) is
  folded into the host-side ew multiply, so device evictions stay raw
  copies.  h8 shares the hT buffer via an AP bitcast (SBUF is the
  binding constraint: w1/w2 bf16 + w2_8 fp8 = 160KB of ~208KB).
  mm1 (c_tile=512): hT[f, c] = gelu(w1.T @ xg); stationary w1[o,f]
      blocks, moving xg (N=512); 1024 MMs total.
  mm2 bf16 (per 128-token block): y = hT.T @ w2; 512 MMs; fp8 blocks:
      y*2^12 = h8.T @ w2_8 DoubleRow (K=256/MM); 256 MMs.
  DMA: weights on the SP HWDGE queue, tokens/outputs on the ACT queue.
"""

import numpy as np
import ml_dtypes

import concourse.mybir as mybir
import concourse.tile as tile
from concourse import bacc
from concourse.bass_utils import run_bass_kernel_spmd

B, S, D = 4, 2048, 1024
E, K, DFF = 8, 2, 4096
T = B * S
C = K * T // E  # 2048 expert capacity
BF16 = ml_dtypes.bfloat16
F8 = ml_dtypes.float8_e4m3  # TRN variant: max +-240
N_CORES = 8

SW2 = 2.0 ** 12        # w2 fp8 scale
P2_INV = 1.0 / SW2     # folded into host ew multiply for fp8-tier slots
QF8 = 8                # 128-slot blocks per expert on the fp8 mm2 path (of 16)

KO1, KO2 = D // 128, DFF // 128  # 8, 32
NFI = DFF // 256                 # 16 k-pairs for fp8 mm2
W1_CH = 16           # f-chunks of w1 (separate tiles -> fine-grained DMA deps)
W1_F = DFF // W1_CH  # 256
W2_CH = 8            # f-chunks of w2
W2_O = KO2 // W2_CH  # 4
W28_CH = 4           # fi-chunks of w2_8
W28_O = NFI // W28_CH  # 4

DR = mybir.MatmulPerfMode.DoubleRow

_NC = None


def _build_nc(c_tile=512, ps1_bufs=4, ps2_bufs=4, y_bufs=3,
              n_iters=1, debug=True):
    nc = bacc.Bacc(None, target_bir_lowering=False, debug=debug)
    f8 = mybir.dt.float8e4
    xgT = nc.dram_tensor("xgT", [D, C], mybir.dt.bfloat16, kind="ExternalInput")
    w1 = nc.dram_tensor("w1", [D, DFF], mybir.dt.bfloat16, kind="ExternalInput")
    w2 = nc.dram_tensor("w2", [DFF, D], mybir.dt.bfloat16, kind="ExternalInput")
    w2_8 = nc.dram_tensor("w2_8", [DFF, D], f8, kind="ExternalInput")
    y = nc.dram_tensor("y", [C, D], mybir.dt.float32, kind="ExternalOutput")

    xgT_v = xgT.rearrange("(o p) c -> p o c", p=128)   # [128, 8, C]
    w1_v = w1.rearrange("(o p) f -> p o f", p=128)     # [128, 8, DFF]
    w2_v = w2.rearrange("(f p) d -> p f d", p=128)     # [128, 32, D]
    w28_v = w2_8.rearrange("(fi j p) d -> p fi j d", p=128, j=2)  # [128,16,2,D]
    n_ct = C // c_tile                                 # 4
    CB = c_tile // 128                                 # 4 c-blocks per c_tile
    n_f8_ct = QF8 // CB                                # fp8 c_tiles (2)

    with tile.TileContext(nc) as tc:
        with (
            tc.tile_pool(name="wpool", bufs=1) as wpool,
            tc.tile_pool(name="xpool", bufs=1) as xpool,
            tc.tile_pool(name="hpool", bufs=1) as hpool,
            tc.tile_pool(name="ypool", bufs=y_bufs) as ypool,
            tc.tile_pool(name="ps1", bufs=ps1_bufs, space="PSUM") as ps1,
            tc.tile_pool(name="ps2", bufs=ps2_bufs, space="PSUM") as ps2,
        ):
            xg_tiles = {}

            def xg_dma(t):
                xg_tiles[t] = xpool.tile(
                    [128, KO1, c_tile], mybir.dt.bfloat16, tag="xg",
                    name=f"xg{t}",
                )
                cs = slice(t * c_tile, (t + 1) * c_tile)
                if t == 0:
                    # split per o-block: mm1(f=0,o) can start on chunk o
                    for o in range(KO1):
                        nc.scalar.dma_start(xg_tiles[t][:, o, :],
                                            xgT_v[:, o, cs])
                else:
                    nc.scalar.dma_start(xg_tiles[t][:], xgT_v[:, :, cs])

            if n_iters == 1:
                xg_dma(0)

            w1_tiles = []
            for ch in range(W1_CH):
                wt = wpool.tile([128, KO1, W1_F], mybir.dt.bfloat16,
                                tag=f"w1_{ch}", name=f"w1t{ch}")
                nc.sync.dma_start(wt[:], w1_v[:, :, ch * W1_F : (ch + 1) * W1_F])
                w1_tiles.append(wt)
            w2_tiles = []
            for ch in range(W2_CH):
                wt = wpool.tile([128, W2_O, D], mybir.dt.bfloat16,
                                tag=f"w2_{ch}", name=f"w2t{ch}")
                nc.sync.dma_start(wt[:], w2_v[:, ch * W2_O : (ch + 1) * W2_O, :])
                w2_tiles.append(wt)
            w28_tiles = []
            for ch in range(W28_CH):
                wt = wpool.tile([128, W28_O, 2, D], f8,
                                tag=f"w28_{ch}", name=f"w28t{ch}")
                nc.sync.dma_start(
                    wt[:], w28_v[:, ch * W28_O : (ch + 1) * W28_O, :, :])
                w28_tiles.append(wt)

            def w1_ap(o, f):
                ch, r = divmod(f, W1_F // 128)
                return w1_tiles[ch][:, o, r * 128 : (r + 1) * 128]

            def w2_ap(f, half):
                ch, r = divmod(f, W2_O)
                return w2_tiles[ch][:, r, half * 512 : (half + 1) * 512]

            def w28_ap(fi, half):
                ch, r = divmod(fi, W28_O)
                return w28_tiles[ch][:, r, :, half * 512 : (half + 1) * 512]

            def body(_=None):
                for t in range(n_ct):
                    if t == 0 and t not in xg_tiles:
                        xg_dma(0)
                    if t + 1 < n_ct:
                        xg_dma(t + 1)  # prefetch next c_tile during this one
                    xg_sb = xg_tiles[t]
                    is_f8 = t < n_f8_ct
                    hT_sb = hpool.tile([128, KO2, c_tile], mybir.dt.bfloat16,
                                       tag="hT", name=f"hT{t}")
                    h8_sb = hT_sb[:].bitcast(f8)  # [128, KO2, 2*c_tile]
                    # mm1: hT[f, c] = gelu(w1.T @ xg)  (bf16, both tiers)
                    for f in range(KO2):
                        psum = ps1.tile([128, c_tile], mybir.dt.float32,
                                        tag="p1")
                        for o in range(KO1):
                            nc.tensor.matmul(
                                psum[:], w1_ap(o, f), xg_sb[:, o, :],
                                start=(o == 0), stop=(o == KO1 - 1),
                            )
                        if is_f8:
                            nc.scalar.activation(
                                h8_sb[:, f, 0:c_tile], psum[:],
                                mybir.ActivationFunctionType.Gelu,
                            )
                        else:
                            nc.scalar.activation(
                                hT_sb[:, f, :], psum[:],
                                mybir.ActivationFunctionType.Gelu,
                            )
                    # mm2: y[c, d] per 128-token block
                    for cb in range(CB):
                        pa = ps2.tile([128, 512], mybir.dt.float32, tag="p2",
                                      name=f"p2a{t}_{cb}")
                        pb = ps2.tile([128, 512], mybir.dt.float32, tag="p2",
                                      name=f"p2b{t}_{cb}")
                        cbs = slice(cb * 128, (cb + 1) * 128)
                        if is_f8:
                            # fp8 DoubleRow: psum = h8.T @ w2_8 = y * 2^12
                            for fi in range(NFI):
                                lhsT = h8_sb[:, 2 * fi : 2 * fi + 2, cbs]
                                first, last = (fi == 0), (fi == NFI - 1)
                                nc.tensor.matmul(
                                    pa[:], lhsT, w28_ap(fi, 0),
                                    start=first, stop=last, perf_mode=DR,
                                )
                                nc.tensor.matmul(
                                    pb[:], lhsT, w28_ap(fi, 1),
                                    start=first, stop=last, perf_mode=DR,
                                )
                        else:
                            for f in range(KO2):
                                lhsT = hT_sb[:, f, cbs]
                                first, last = (f == 0), (f == KO2 - 1)
                                nc.tensor.matmul(
                                    pa[:], lhsT, w2_ap(f, 0),
                                    start=first, stop=last,
                                )
                                nc.tensor.matmul(
                                    pb[:], lhsT, w2_ap(f, 1),
                                    start=first, stop=last,
                                )
                        ya = ypool.tile([128, 512], mybir.dt.float32, tag="y",
                                        name=f"ya{t}_{cb}")
                        yb = ypool.tile([128, 512], mybir.dt.float32, tag="y",
                                        name=f"yb{t}_{cb}")
                        # split evictions across DVE and ACT (raw copies; the
                        # fp8-tier 2^-12 descale happens on the host)
                        nc.vector.tensor_copy(ya[:], pa[:])
                        nc.scalar.copy(yb[:], pb[:])
                        r0 = t * c_tile + cb * 128
                        if t == n_ct - 1 and cb == CB - 1:
                            # last block: split the DMA across both queues to
                            # shorten the end-of-kernel drain
                            nc.scalar.dma_start(y[r0 : r0 + 128, 0:512],
                                                ya[:])
                            nc.sync.dma_start(y[r0 : r0 + 128, 512:1024],
                                              yb[:])
                        else:
                            nc.scalar.dma_start(y[r0 : r0 + 128, 0:512],
                                                ya[:])
                            nc.scalar.dma_start(y[r0 : r0 + 128, 512:1024],
                                                yb[:])

            if n_iters == 1:
                body()
            else:
                with tc.For_i(0, n_iters, 1):
                    body()
    nc.compile()
    return nc


def _get_nc():
    global _NC
    if _NC is None:
        _NC = _build_nc()
    return _NC


def _route(x, wr):
    """Replicates the reference router exactly (fp32 numpy), then permutes
    each expert's kept slots into ascending-ew order (low-ew slots first =
    the fp8 mm2 tier)."""
    xt = np.transpose(x, (1, 0, 2)).reshape(T, D)  # [T, D] fp32
    logits = xt.astype(np.float32) @ wr.astype(np.float32)  # [T, E]
    m = logits.max(axis=-1, keepdims=True)
    p = np.exp(logits - m, dtype=np.float32)
    p /= p.sum(axis=-1, keepdims=True)
    top1 = np.argmax(p, axis=-1)
    p_masked = p.copy()
    p_masked[np.arange(T), top1] = -np.inf
    top2 = np.argmax(p_masked, axis=-1)
    eidx = np.stack([top1, top2], axis=1)  # [T, K]
    ew = np.take_along_axis(p, eidx, axis=1).astype(np.float32)  # [T, K]

    flat_e = eidx.reshape(-1)
    order = np.argsort(flat_e, kind="stable")
    sorted_e = flat_e[order]
    hist = np.bincount(flat_e, minlength=E)
    starts = np.cumsum(hist) - hist
    pos = np.arange(T * K) - starts[sorted_e]
    keep = pos < C
    slot = np.where(keep, sorted_e * C + pos, E * C)
    token = order // K

    # permute kept slots within each expert by ascending ew (padding slots
    # have ew=0 and sort first, landing harmlessly in the fp8 tier)
    w_disp = ew.reshape(-1)[order]
    ew_slot = np.zeros(E * C, np.float32)
    ew_slot[slot[keep]] = w_disp[keep]
    slot_perm = np.empty(E * C + 1, np.int64)
    slot_perm[E * C] = E * C
    for e in range(E):
        seg = ew_slot[e * C : (e + 1) * C]
        rank = np.argsort(seg, kind="stable")
        newpos = np.empty(C, np.int64)
        newpos[rank] = np.arange(C)
        slot_perm[e * C : (e + 1) * C] = e * C + newpos
    slot = slot_perm[slot]
    return xt, ew, order, keep, slot, token


def _make_in_maps(x, wr, w1, w2):
    xt, ew, order, keep, slot, token = _route(x, wr)
    slot_token = np.zeros(E * C, np.int64)
    slot_token[slot[keep]] = token[keep]
    xT_bf = np.ascontiguousarray(xt.T.astype(BF16))  # [D, T]
    in_maps = []
    for e in range(E):
        idx = slot_token[e * C : (e + 1) * C]
        w2e = w2[e]
        w2_8 = np.clip(w2e * SW2, -240.0, 240.0).astype(F8)
        in_maps.append(
            {
                "xgT": np.ascontiguousarray(xT_bf[:, idx]),
                "w1": np.ascontiguousarray(w1[e].astype(BF16)),
                "w2": np.ascontiguousarray(w2e.astype(BF16)),
                "w2_8": np.ascontiguousarray(w2_8),
            }
        )
    return in_maps, (ew, order, keep, slot)


def kernel(x, wr, w1, w2):
    nc = _get_nc()
    in_maps, (ew, order, keep, slot) = _make_in_maps(x, wr, w1, w2)

    def _run_device():
        res = run_bass_kernel_spmd(nc, in_maps, core_ids=list(range(N_CORES)))
        # force materialization here so device errors surface inside the retry
        return [np.asarray(res.results[e]["y"], np.float32) for e in range(E)]

    try:
        Ys = _run_device()
    except Exception:
        # transient device errors (wedged core) usually clear on retry
        Ys = _run_device()

    # --- combine: weighted scatter back to tokens ---
    Y = np.empty((E * C, D), np.float32)
    nf8 = 128 * QF8
    for e in range(E):
        Y[e * C : (e + 1) * C] = Ys[e]
        # fp8-tier blocks hold y * 2^12 (raw psum); descale here
        Y[e * C : e * C + nf8] *= P2_INV

    inv = np.empty(T * K, np.int64)
    inv[order] = np.arange(T * K)
    slot_tk = slot[inv].reshape(T, K)
    keep_tk = keep[inv].reshape(T, K)

    out_flat = np.zeros((T, D), np.float32)
    for k in range(K):
        sl = np.clip(slot_tk[:, k], 0, E * C - 1)
        contrib = Y[sl] * ew[:, k : k + 1]
        contrib[~keep_tk[:, k]] = 0.0
        out_flat += contrib
    return np.ascontiguousarray(
        out_flat.reshape(S, B, D).transpose(1, 0, 2)
    ).astype(np.float32)


# ---------------------------------------------------------------------------
# Benchmark helper (used by test.py; not part of the grading contract).
# ---------------------------------------------------------------------------


def make_bench(in_maps):
    import jax
    from jax.experimental.shard_map import shard_map
    from jax.sharding import Mesh, PartitionSpec, NamedSharding
    from concourse.bass2jax import (
        _bass_exec_p,
        install_neuronx_cc_hook,
        partition_id_tensor,
    )

    nc = _NC if _NC is not None else _get_nc()
    install_neuronx_cc_hook()
    partition_name = nc.partition_id_tensor.name if nc.partition_id_tensor else None

    in_names, out_names, out_avals, zero_outs = [], [], [], []
    for alloc in nc.m.functions[0].allocations:
        if not isinstance(alloc, mybir.MemoryLocationSet):
            continue
        name = alloc.memorylocations[0].name
        if alloc.kind == "ExternalInput":
            if name != partition_name:
                in_names.append(name)
        elif alloc.kind == "ExternalOutput":
            shape = tuple(alloc.tensor_shape)
            dtype = mybir.dt.np(alloc.dtype)
            out_avals.append(jax.core.ShapedArray(shape, dtype))
            zero_outs.append(np.zeros(shape, dtype))
            out_names.append(name)
    n_params = len(in_names)
    all_in_names = list(in_names) + list(out_names)
    if partition_name is not None:
        all_in_names.append(partition_name)
    if nc.dbg_addr is not None:
        dbg_zero = np.zeros((1, 2), np.uint32)
        in_maps = [{**m, nc.dbg_addr.name: dbg_zero} for m in in_maps]

    def _body(*args):
        operands = list(args)
        if partition_name is not None:
            operands.append(partition_id_tensor())
        outs = _bass_exec_p.bind(
            *operands,
            out_avals=tuple(out_avals),
            in_names=tuple(all_in_names),
            out_names=tuple(out_names),
            lowering_input_output_aliases=(),
            sim_require_finite=True,
            sim_require_nnan=True,
            nc=nc,
        )
        return tuple(outs)

    devices = jax.devices()[:N_CORES]
    mesh = Mesh(np.asarray(devices), ("core",))
    n_outs = len(out_names)
    in_specs = (PartitionSpec("core"),) * (n_params + n_outs)
    out_specs = (PartitionSpec("core"),) * n_outs
    fn = jax.jit(
        shard_map(_body, mesh=mesh, in_specs=in_specs, out_specs=out_specs,
                  check_rep=False),
        keep_unused=True,
    )
    concat_in = [
        np.concatenate([np.asarray(in_maps[c][name]) for c in range(N_CORES)],
                       axis=0)
        for name in in_names
    ]
    concat_zeros = [
        np.zeros((N_CORES * z.shape[0], *z.shape[1:]), z.dtype)
        for z in zero_outs
    ]
    shard = NamedSharding(mesh, PartitionSpec("core"))
    args = [jax.device_put(a, shard) for a in concat_in + concat_zeros]
    return fn, args, out_names


def benchmark(in_maps, iters=20, warmup=3):
    import time
    import jax

    fn, args, out_names = make_bench(in_maps)
    for _ in range(warmup):
        out = fn(*args)
        jax.block_until_ready(out)
    times = []
    for _ in range(iters):
        t0 = time.perf_counter()
        out = fn(*args)
        jax.block_until_ready(out)
        times.append(time.perf_counter() - t0)
    return min(times), sorted(times)[len(times) // 2], out


# revision 8
# speedup vs baseline: 1.1746x; 1.0226x over previous
"""MegablockMoE kernel for 8 Trainium2 NeuronCores.

Expert-parallel (one expert per core); router/dispatch on host.

v7: bf16 mm1 everywhere + router-weight-tiered mm2.
  Within each expert, slots are host-permuted in ascending router-weight
  (ew) order.  The lowest-ew half of the slots (Q=8 of 16 blocks, which
  carries only ~15% of the ew^2-weighted output power) runs mm2 in
  fp8(e4m3) DoubleRow at 2x instruction throughput: stationary h8 =
  fp8(gelu(psum)) pairs over f, moving w2_8 = fp8(w2 * 2^12).  The
  remaining high-ew half runs mm2 in bf16 exactly as v5.  Measured
  rel-l2 error 1.5e-2 (gate 2e-2); the fp8 psum descale (2^-12) is
  folded into the host-side ew multiply, so device evictions stay raw
  copies.  h8 shares the hT buffer via an AP bitcast (SBUF is the
  binding constraint: w1/w2 bf16 + w2_8 fp8 = 160KB of ~208KB).
  mm1 (c_tile=512): hT[f, c] = gelu(w1.T @ xg); stationary w1[o,f]
      blocks, moving xg (N=512); 1024 MMs total.
  mm2 bf16 (per 128-token block): y = hT.T @ w2; 512 MMs; fp8 blocks:
      y*2^12 = h8.T @ w2_8 DoubleRow (K=256/MM); 256 MMs.
  DMA: weights on the SP HWDGE queue, tokens/outputs on the ACT queue.
"""

import numpy as np
import ml_dtypes

import concourse.mybir as mybir
import concourse.tile as tile
from concourse import bacc
from concourse.bass_utils import run_bass_kernel_spmd

B, S, D = 4, 2048, 1024
E, K, DFF = 8, 2, 4096
T = B * S
C = K * T // E  # 2048 expert capacity
BF16 = ml_dtypes.bfloat16
F8 = ml_dtypes.float8_e4m3  # TRN variant: max +-240
N_CORES = 8

SW2 = 2.0 ** 12        # w2 fp8 scale
P2_INV = 1.0 / SW2     # folded into host ew multiply for fp8-tier slots
QF8 = 9                # 128-slot blocks per expert on the fp8 mm2 path (of 16)

KO1, KO2 = D // 128, DFF // 128  # 8, 32
NFI = DFF // 256                 # 16 k-pairs for fp8 mm2
W1_CH = 16           # f-chunks of w1 (separate tiles -> fine-grained DMA deps)
W1_F = DFF // W1_CH  # 256
W2_CH = 8            # f-chunks of w2
W2_O = KO2 // W2_CH  # 4
W28_CH = 4           # fi-chunks of w2_8
W28_O = NFI // W28_CH  # 4

DR = mybir.MatmulPerfMode.DoubleRow

_NC = None


def _build_nc(c_tile=512, ps1_bufs=4, ps2_bufs=4, y_bufs=3,
              n_iters=1, debug=True):
    nc = bacc.Bacc(None, target_bir_lowering=False, debug=debug)
    f8 = mybir.dt.float8e4
    xgT = nc.dram_tensor("xgT", [D, C], mybir.dt.bfloat16, kind="ExternalInput")
    w1 = nc.dram_tensor("w1", [D, DFF], mybir.dt.bfloat16, kind="ExternalInput")
    w2 = nc.dram_tensor("w2", [DFF, D], mybir.dt.bfloat16, kind="ExternalInput")
    w2_8 = nc.dram_tensor("w2_8", [DFF, D], f8, kind="ExternalInput")
    y = nc.dram_tensor("y", [C, D], mybir.dt.float32, kind="ExternalOutput")

    xgT_v = xgT.rearrange("(o p) c -> p o c", p=128)   # [128, 8, C]
    w1_v = w1.rearrange("(o p) f -> p o f", p=128)     # [128, 8, DFF]
    w2_v = w2.rearrange("(f p) d -> p f d", p=128)     # [128, 32, D]
    w28_v = w2_8.rearrange("(fi j p) d -> p fi j d", p=128, j=2)  # [128,16,2,D]
    n_ct = C // c_tile                                 # 4
    CB = c_tile // 128                                 # 4 c-blocks per c_tile
    # per c_tile: number of leading 128-blocks on the fp8 mm2 path
    f8_blocks = [max(0, min(CB, QF8 - t * CB)) for t in range(n_ct)]

    with tile.TileContext(nc) as tc:
        with (
            tc.tile_pool(name="wpool", bufs=1) as wpool,
            tc.tile_pool(name="xpool", bufs=1) as xpool,
            tc.tile_pool(name="hpool", bufs=1) as hpool,
            tc.tile_pool(name="ypool", bufs=y_bufs) as ypool,
            tc.tile_pool(name="ps1", bufs=ps1_bufs, space="PSUM") as ps1,
            tc.tile_pool(name="ps2", bufs=ps2_bufs, space="PSUM") as ps2,
        ):
            xg_tiles = {}

            def xg_dma(t):
                xg_tiles[t] = xpool.tile(
                    [128, KO1, c_tile], mybir.dt.bfloat16, tag="xg",
                    name=f"xg{t}",
                )
                cs = slice(t * c_tile, (t + 1) * c_tile)
                # xg rides the SP queue (idle after the one-time weight
                # load) so it never queues behind the y-output DMAs
                if t == 0:
                    # split per o-block: mm1(f=0,o) can start on chunk o
                    for o in range(KO1):
                        nc.sync.dma_start(xg_tiles[t][:, o, :],
                                          xgT_v[:, o, cs])
                else:
                    nc.sync.dma_start(xg_tiles[t][:], xgT_v[:, :, cs])

            if n_iters == 1:
                xg_dma(0)

            w1_tiles = []
            for ch in range(W1_CH):
                wt = wpool.tile([128, KO1, W1_F], mybir.dt.bfloat16,
                                tag=f"w1_{ch}", name=f"w1t{ch}")
                nc.sync.dma_start(wt[:], w1_v[:, :, ch * W1_F : (ch + 1) * W1_F])
                w1_tiles.append(wt)
            w2_tiles = []
            for ch in range(W2_CH):
                wt = wpool.tile([128, W2_O, D], mybir.dt.bfloat16,
                                tag=f"w2_{ch}", name=f"w2t{ch}")
                nc.sync.dma_start(wt[:], w2_v[:, ch * W2_O : (ch + 1) * W2_O, :])
                w2_tiles.append(wt)
            w28_tiles = []
            for ch in range(W28_CH):
                wt = wpool.tile([128, W28_O, 2, D], f8,
                                tag=f"w28_{ch}", name=f"w28t{ch}")
                nc.sync.dma_start(
                    wt[:], w28_v[:, ch * W28_O : (ch + 1) * W28_O, :, :])
                w28_tiles.append(wt)

            def w1_ap(o, f):
                ch, r = divmod(f, W1_F // 128)
                return w1_tiles[ch][:, o, r * 128 : (r + 1) * 128]

            def w2_ap(f, half):
                ch, r = divmod(f, W2_O)
                return w2_tiles[ch][:, r, half * 512 : (half + 1) * 512]

            def w28_ap(fi, half):
                ch, r = divmod(fi, W28_O)
                return w28_tiles[ch][:, r, :, half * 512 : (half + 1) * 512]

            def body(_=None):
                for t in range(n_ct):
                    if t == 0 and t not in xg_tiles:
                        xg_dma(0)
                    if t + 1 < n_ct:
                        xg_dma(t + 1)  # prefetch next c_tile during this one
                    xg_sb = xg_tiles[t]
                    nb8 = f8_blocks[t]  # leading fp8 blocks in this c_tile
                    nc8 = nb8 * 128
                    hT_sb = hpool.tile([128, KO2, c_tile], mybir.dt.bfloat16,
                                       tag="hT", name=f"hT{t}")
                    h8_sb = hT_sb[:].bitcast(f8)  # [128, KO2, 2*c_tile]
                    # mm1: hT[f, c] = gelu(w1.T @ xg)  (bf16, both tiers);
                    # eviction dtype split at the fp8/bf16 block boundary
                    # (byte ranges: fp8 cols [0,nc8) vs bf16 cols [nc8,...)
                    # do not overlap inside the shared buffer)
                    for f in range(KO2):
                        psum = ps1.tile([128, c_tile], mybir.dt.float32,
                                        tag="p1")
                        for o in range(KO1):
                            nc.tensor.matmul(
                                psum[:], w1_ap(o, f), xg_sb[:, o, :],
                                start=(o == 0), stop=(o == KO1 - 1),
                            )
                        if nb8 > 0:
                            nc.scalar.activation(
                                h8_sb[:, f, 0:nc8], psum[:, 0:nc8],
                                mybir.ActivationFunctionType.Gelu,
                            )
                        if nb8 < CB:
                            nc.scalar.activation(
                                hT_sb[:, f, nc8:c_tile], psum[:, nc8:c_tile],
                                mybir.ActivationFunctionType.Gelu,
                            )
                    # mm2: y[c, d] per 128-token block
                    for cb in range(CB):
                        pa = ps2.tile([128, 512], mybir.dt.float32, tag="p2",
                                      name=f"p2a{t}_{cb}")
                        pb = ps2.tile([128, 512], mybir.dt.float32, tag="p2",
                                      name=f"p2b{t}_{cb}")
                        cbs = slice(cb * 128, (cb + 1) * 128)
                        if cb < nb8:
                            # fp8 DoubleRow: psum = h8.T @ w2_8 = y * 2^12
                            for fi in range(NFI):
                                lhsT = h8_sb[:, 2 * fi : 2 * fi + 2, cbs]
                                first, last = (fi == 0), (fi == NFI - 1)
                                nc.tensor.matmul(
                                    pa[:], lhsT, w28_ap(fi, 0),
                                    start=first, stop=last, perf_mode=DR,
                                )
                                nc.tensor.matmul(
                                    pb[:], lhsT, w28_ap(fi, 1),
                                    start=first, stop=last, perf_mode=DR,
                                )
                        else:
                            for f in range(KO2):
                                lhsT = hT_sb[:, f, cbs]
                                first, last = (f == 0), (f == KO2 - 1)
                                nc.tensor.matmul(
                                    pa[:], lhsT, w2_ap(f, 0),
                                    start=first, stop=last,
                                )
                                nc.tensor.matmul(
                                    pb[:], lhsT, w2_ap(f, 1),
                                    start=first, stop=last,
                                )
                        ya = ypool.tile([128, 512], mybir.dt.float32, tag="y",
                                        name=f"ya{t}_{cb}")
                        yb = ypool.tile([128, 512], mybir.dt.float32, tag="y",
                                        name=f"yb{t}_{cb}")
                        # split evictions across DVE and ACT (raw copies; the
                        # fp8-tier 2^-12 descale happens on the host)
                        nc.vector.tensor_copy(ya[:], pa[:])
                        nc.scalar.copy(yb[:], pb[:])
                        r0 = t * c_tile + cb * 128
                        if t == n_ct - 1 and cb == CB - 1:
                            # last block: split the DMA across both queues to
                            # shorten the end-of-kernel drain
                            nc.scalar.dma_start(y[r0 : r0 + 128, 0:512],
                                                ya[:])
                            nc.sync.dma_start(y[r0 : r0 + 128, 512:1024],
                                              yb[:])
                        else:
                            nc.scalar.dma_start(y[r0 : r0 + 128, 0:512],
                                                ya[:])
                            nc.scalar.dma_start(y[r0 : r0 + 128, 512:1024],
                                                yb[:])

            if n_iters == 1:
                body()
            else:
                with tc.For_i(0, n_iters, 1):
                    body()
    nc.compile()
    return nc


def _get_nc():
    global _NC
    if _NC is None:
        _NC = _build_nc()
    return _NC


def _route(x, wr):
    """Replicates the reference router exactly (fp32 numpy), then permutes
    each expert's kept slots into ascending-ew order (low-ew slots first =
    the fp8 mm2 tier)."""
    xt = np.transpose(x, (1, 0, 2)).reshape(T, D)  # [T, D] fp32
    logits = xt.astype(np.float32) @ wr.astype(np.float32)  # [T, E]
    m = logits.max(axis=-1, keepdims=True)
    p = np.exp(logits - m, dtype=np.float32)
    p /= p.sum(axis=-1, keepdims=True)
    top1 = np.argmax(p, axis=-1)
    p_masked = p.copy()
    p_masked[np.arange(T), top1] = -np.inf
    top2 = np.argmax(p_masked, axis=-1)
    eidx = np.stack([top1, top2], axis=1)  # [T, K]
    ew = np.take_along_axis(p, eidx, axis=1).astype(np.float32)  # [T, K]

    flat_e = eidx.reshape(-1)
    order = np.argsort(flat_e, kind="stable")
    sorted_e = flat_e[order]
    hist = np.bincount(flat_e, minlength=E)
    starts = np.cumsum(hist) - hist
    pos = np.arange(T * K) - starts[sorted_e]
    keep = pos < C
    slot = np.where(keep, sorted_e * C + pos, E * C)
    token = order // K

    # permute kept slots within each expert by ascending ew (padding slots
    # have ew=0 and sort first, landing harmlessly in the fp8 tier)
    w_disp = ew.reshape(-1)[order]
    ew_slot = np.zeros(E * C, np.float32)
    ew_slot[slot[keep]] = w_disp[keep]
    slot_perm = np.empty(E * C + 1, np.int64)
    slot_perm[E * C] = E * C
    for e in range(E):
        seg = ew_slot[e * C : (e + 1) * C]
        rank = np.argsort(seg, kind="stable")
        newpos = np.empty(C, np.int64)
        newpos[rank] = np.arange(C)
        slot_perm[e * C : (e + 1) * C] = e * C + newpos
    slot = slot_perm[slot]
    return xt, ew, order, keep, slot, token


def _make_in_maps(x, wr, w1, w2):
    xt, ew, order, keep, slot, token = _route(x, wr)
    slot_token = np.zeros(E * C, np.int64)
    slot_token[slot[keep]] = token[keep]
    xT_bf = np.ascontiguousarray(xt.T.astype(BF16))  # [D, T]
    in_maps = []
    for e in range(E):
        idx = slot_token[e * C : (e + 1) * C]
        w2e = w2[e]
        w2_8 = np.clip(w2e * SW2, -240.0, 240.0).astype(F8)
        in_maps.append(
            {
                "xgT": np.ascontiguousarray(xT_bf[:, idx]),
                "w1": np.ascontiguousarray(w1[e].astype(BF16)),
                "w2": np.ascontiguousarray(w2e.astype(BF16)),
                "w2_8": np.ascontiguousarray(w2_8),
            }
        )
    return in_maps, (ew, order, keep, slot)


def kernel(x, wr, w1, w2):
    nc = _get_nc()
    in_maps, (ew, order, keep, slot) = _make_in_maps(x, wr, w1, w2)

    def _run_device():
        res = run_bass_kernel_spmd(nc, in_maps, core_ids=list(range(N_CORES)))
        # force materialization here so device errors surface inside the retry
        return [np.asarray(res.results[e]["y"], np.float32) for e in range(E)]

    try:
        Ys = _run_device()
    except Exception:
        # transient device errors (wedged core) usually clear on retry
        Ys = _run_device()

    # --- combine: weighted scatter back to tokens ---
    Y = np.empty((E * C, D), np.float32)
    nf8 = 128 * QF8
    for e in range(E):
        Y[e * C : (e + 1) * C] = Ys[e]
        # fp8-tier blocks hold y * 2^12 (raw psum); descale here
        Y[e * C : e * C + nf8] *= P2_INV

    inv = np.empty(T * K, np.int64)
    inv[order] = np.arange(T * K)
    slot_tk = slot[inv].reshape(T, K)
    keep_tk = keep[inv].reshape(T, K)

    out_flat = np.zeros((T, D), np.float32)
    for k in range(K):
        sl = np.clip(slot_tk[:, k], 0, E * C - 1)
        contrib = Y[sl] * ew[:, k : k + 1]
        contrib[~keep_tk[:, k]] = 0.0
        out_flat += contrib
    return np.ascontiguousarray(
        out_flat.reshape(S, B, D).transpose(1, 0, 2)
    ).astype(np.float32)


# ---------------------------------------------------------------------------
# Benchmark helper (used by test.py; not part of the grading contract).
# ---------------------------------------------------------------------------


def make_bench(in_maps):
    import jax
    from jax.experimental.shard_map import shard_map
    from jax.sharding import Mesh, PartitionSpec, NamedSharding
    from concourse.bass2jax import (
        _bass_exec_p,
        install_neuronx_cc_hook,
        partition_id_tensor,
    )

    nc = _NC if _NC is not None else _get_nc()
    install_neuronx_cc_hook()
    partition_name = nc.partition_id_tensor.name if nc.partition_id_tensor else None

    in_names, out_names, out_avals, zero_outs = [], [], [], []
    for alloc in nc.m.functions[0].allocations:
        if not isinstance(alloc, mybir.MemoryLocationSet):
            continue
        name = alloc.memorylocations[0].name
        if alloc.kind == "ExternalInput":
            if name != partition_name:
                in_names.append(name)
        elif alloc.kind == "ExternalOutput":
            shape = tuple(alloc.tensor_shape)
            dtype = mybir.dt.np(alloc.dtype)
            out_avals.append(jax.core.ShapedArray(shape, dtype))
            zero_outs.append(np.zeros(shape, dtype))
            out_names.append(name)
    n_params = len(in_names)
    all_in_names = list(in_names) + list(out_names)
    if partition_name is not None:
        all_in_names.append(partition_name)
    if nc.dbg_addr is not None:
        dbg_zero = np.zeros((1, 2), np.uint32)
        in_maps = [{**m, nc.dbg_addr.name: dbg_zero} for m in in_maps]

    def _body(*args):
        operands = list(args)
        if partition_name is not None:
            operands.append(partition_id_tensor())
        outs = _bass_exec_p.bind(
            *operands,
            out_avals=tuple(out_avals),
            in_names=tuple(all_in_names),
            out_names=tuple(out_names),
            lowering_input_output_aliases=(),
            sim_require_finite=True,
            sim_require_nnan=True,
            nc=nc,
        )
        return tuple(outs)

    devices = jax.devices()[:N_CORES]
    mesh = Mesh(np.asarray(devices), ("core",))
    n_outs = len(out_names)
    in_specs = (PartitionSpec("core"),) * (n_params + n_outs)
    out_specs = (PartitionSpec("core"),) * n_outs
    fn = jax.jit(
        shard_map(_body, mesh=mesh, in_specs=in_specs, out_specs=out_specs,
                  check_rep=False),
        keep_unused=True,
    )
    concat_in = [
        np.concatenate([np.asarray(in_maps[c][name]) for c in range(N_CORES)],
                       axis=0)
        for name in in_names
    ]
    concat_zeros = [
        np.zeros((N_CORES * z.shape[0], *z.shape[1:]), z.dtype)
        for z in zero_outs
    ]
    shard = NamedSharding(mesh, PartitionSpec("core"))
    args = [jax.device_put(a, shard) for a in concat_in + concat_zeros]
    return fn, args, out_names


def benchmark(in_maps, iters=20, warmup=3):
    import time
    import jax

    fn, args, out_names = make_bench(in_maps)
    for _ in range(warmup):
        out = fn(*args)
        jax.block_until_ready(out)
    times = []
    for _ in range(iters):
        t0 = time.perf_counter()
        out = fn(*args)
        jax.block_until_ready(out)
        times.append(time.perf_counter() - t0)
    return min(times), sorted(times)[len(times) // 2], out


# revision 11
# speedup vs baseline: 1.1833x; 1.0074x over previous
"""MegablockMoE kernel for 8 Trainium2 NeuronCores.

Expert-parallel (one expert per core); router/dispatch on host.

v9: bf16 mm1 everywhere + router-weight-tiered mm2.
  Within each expert, slots are host-permuted in ascending router-weight
  (ew) order.  The lowest-ew 9 of 16 blocks (which carry only ~17% of
  the ew^2-weighted output power) run mm2 in fp8(e4m3) DoubleRow at 2x
  instruction throughput: stationary h8 = fp8(gelu(psum)) pairs over f,
  moving w2_8 = fp8(w2 * 2^12).  The remaining high-ew blocks run mm2
  in bf16 exactly as v5 (block 8 makes c_tile 2 mixed: its gelu
  eviction is split by column range).  Measured rel-l2 error 1.70e-2
  (gate 2e-2); the fp8 psum descale (2^-12) is folded into the
  host-side ew multiply, so device evictions stay raw copies.  h8
  shares the hT buffer via an AP bitcast (SBUF is the binding
  constraint: w1/w2 bf16 + w2_8 fp8 = 160KB of ~208KB; fp8 cannot go
  further: DoubleRow is 2x per instruction, so any residual-compensated
  scheme >= 2 terms is no faster than bf16, and plain fp8 everywhere
  measures 6.1e-2 rel error).
  mm1 (c_tile=512): hT[f, c] = gelu(w1.T @ xg); stationary w1[o,f]
      blocks, moving xg (N=512); 1024 MMs total.
  mm2 bf16 (per 128-token block): y = hT.T @ w2; 512 MMs; fp8 blocks:
      y*2^12 = h8.T @ w2_8 DoubleRow (K=256/MM); 256 MMs.
  DMA: weights on the SP HWDGE queue, tokens/outputs on the ACT queue.
"""

import numpy as np
import ml_dtypes

import concourse.mybir as mybir
import concourse.tile as tile
from concourse import bacc
from concourse.bass_utils import run_bass_kernel_spmd

B, S, D = 4, 2048, 1024
E, K, DFF = 8, 2, 4096
T = B * S
C = K * T // E  # 2048 expert capacity
BF16 = ml_dtypes.bfloat16
F8 = ml_dtypes.float8_e4m3  # TRN variant: max +-240
N_CORES = 8

SW2 = 2.0 ** 12        # w2 fp8 scale
P2_INV = 1.0 / SW2     # folded into host ew multiply for fp8-tier slots
QF8 = 9                # 128-slot blocks per expert on the fp8 mm2 path (of 16)

KO1, KO2 = D // 128, DFF // 128  # 8, 32
NFI = DFF // 256                 # 16 k-pairs for fp8 mm2
W1_CH = 16           # f-chunks of w1 (separate tiles -> fine-grained DMA deps)
W1_F = DFF // W1_CH  # 256
W2_CH = 8            # f-chunks of w2
W2_O = KO2 // W2_CH  # 4
W28_CH = 4           # fi-chunks of w2_8
W28_O = NFI // W28_CH  # 4

DR = mybir.MatmulPerfMode.DoubleRow

_NC = None


def _build_nc(c_tile=512, ps1_bufs=4, ps2_bufs=4, y_bufs=3,
              n_iters=1, debug=True):
    nc = bacc.Bacc(None, target_bir_lowering=False, debug=debug)
    f8 = mybir.dt.float8e4
    xgT = nc.dram_tensor("xgT", [D, C], mybir.dt.bfloat16, kind="ExternalInput")
    w1 = nc.dram_tensor("w1", [D, DFF], mybir.dt.bfloat16, kind="ExternalInput")
    w2 = nc.dram_tensor("w2", [DFF, D], mybir.dt.bfloat16, kind="ExternalInput")
    w2_8 = nc.dram_tensor("w2_8", [DFF, D], f8, kind="ExternalInput")
    y = nc.dram_tensor("y", [C, D], mybir.dt.float32, kind="ExternalOutput")

    xgT_v = xgT.rearrange("(o p) c -> p o c", p=128)   # [128, 8, C]
    w1_v = w1.rearrange("(o p) f -> p o f", p=128)     # [128, 8, DFF]
    w2_v = w2.rearrange("(f p) d -> p f d", p=128)     # [128, 32, D]
    w28_v = w2_8.rearrange("(fi j p) d -> p fi j d", p=128, j=2)  # [128,16,2,D]
    n_ct = C // c_tile                                 # 4
    CB = c_tile // 128                                 # 4 c-blocks per c_tile
    # per c_tile: number of leading 128-blocks on the fp8 mm2 path
    f8_blocks = [max(0, min(CB, QF8 - t * CB)) for t in range(n_ct)]

    with tile.TileContext(nc) as tc:
        with (
            tc.tile_pool(name="wpool", bufs=1) as wpool,
            tc.tile_pool(name="xpool", bufs=1) as xpool,
            tc.tile_pool(name="hpool", bufs=1) as hpool,
            tc.tile_pool(name="ypool", bufs=y_bufs) as ypool,
            tc.tile_pool(name="ps1", bufs=ps1_bufs, space="PSUM") as ps1,
            tc.tile_pool(name="ps2", bufs=ps2_bufs, space="PSUM") as ps2,
        ):
            xg_tiles = {}

            def xg_dma(t):
                xg_tiles[t] = xpool.tile(
                    [128, KO1, c_tile], mybir.dt.bfloat16, tag="xg",
                    name=f"xg{t}",
                )
                cs = slice(t * c_tile, (t + 1) * c_tile)
                # xg rides the SP queue (idle after the one-time weight
                # load) so it never queues behind the y-output DMAs
                if t == 0:
                    # split per o-block: mm1(f=0,o) can start on chunk o
                    for o in range(KO1):
                        nc.sync.dma_start(xg_tiles[t][:, o, :],
                                          xgT_v[:, o, cs])
                else:
                    nc.sync.dma_start(xg_tiles[t][:], xgT_v[:, :, cs])

            if n_iters == 1:
                xg_dma(0)

            w1_tiles = []
            for ch in range(W1_CH):
                wt = wpool.tile([128, KO1, W1_F], mybir.dt.bfloat16,
                                tag=f"w1_{ch}", name=f"w1t{ch}")
                nc.sync.dma_start(wt[:], w1_v[:, :, ch * W1_F : (ch + 1) * W1_F])
                w1_tiles.append(wt)
            w2_tiles = []
            for ch in range(W2_CH):
                wt = wpool.tile([128, W2_O, D], mybir.dt.bfloat16,
                                tag=f"w2_{ch}", name=f"w2t{ch}")
                nc.sync.dma_start(wt[:], w2_v[:, ch * W2_O : (ch + 1) * W2_O, :])
                w2_tiles.append(wt)
            w28_tiles = []
            for ch in range(W28_CH):
                wt = wpool.tile([128, W28_O, 2, D], f8,
                                tag=f"w28_{ch}", name=f"w28t{ch}")
                nc.sync.dma_start(
                    wt[:], w28_v[:, ch * W28_O : (ch + 1) * W28_O, :, :])
                w28_tiles.append(wt)

            def w1_ap(o, f):
                ch, r = divmod(f, W1_F // 128)
                return w1_tiles[ch][:, o, r * 128 : (r + 1) * 128]

            def w2_ap(f, half):
                ch, r = divmod(f, W2_O)
                return w2_tiles[ch][:, r, half * 512 : (half + 1) * 512]

            def w28_ap(fi, half):
                ch, r = divmod(fi, W28_O)
                return w28_tiles[ch][:, r, :, half * 512 : (half + 1) * 512]

            def body(_=None):
                for t in range(n_ct):
                    if t == 0 and t not in xg_tiles:
                        xg_dma(0)
                    if t + 1 < n_ct:
                        xg_dma(t + 1)  # prefetch next c_tile during this one
                    xg_sb = xg_tiles[t]
                    nb8 = f8_blocks[t]  # leading fp8 blocks in this c_tile
                    nc8 = nb8 * 128
                    hT_sb = hpool.tile([128, KO2, c_tile], mybir.dt.bfloat16,
                                       tag="hT", name=f"hT{t}")
                    h8_sb = hT_sb[:].bitcast(f8)  # [128, KO2, 2*c_tile]
                    # mm1: hT[f, c] = gelu(w1.T @ xg)  (bf16, both tiers);
                    # eviction dtype split at the fp8/bf16 block boundary
                    # (byte ranges: fp8 cols [0,nc8) vs bf16 cols [nc8,...)
                    # do not overlap inside the shared buffer)
                    for f in range(KO2):
                        psum = ps1.tile([128, c_tile], mybir.dt.float32,
                                        tag="p1")
                        for o in range(KO1):
                            nc.tensor.matmul(
                                psum[:], w1_ap(o, f), xg_sb[:, o, :],
                                start=(o == 0), stop=(o == KO1 - 1),
                            )
                        if nb8 > 0:
                            nc.scalar.activation(
                                h8_sb[:, f, 0:nc8], psum[:, 0:nc8],
                                mybir.ActivationFunctionType.Gelu,
                            )
                        if nb8 < CB:
                            nc.scalar.activation(
                                hT_sb[:, f, nc8:c_tile], psum[:, nc8:c_tile],
                                mybir.ActivationFunctionType.Gelu,
                            )
                    # mm2: y[c, d] per 128-token block
                    for cb in range(CB):
                        pa = ps2.tile([128, 512], mybir.dt.float32, tag="p2",
                                      name=f"p2a{t}_{cb}")
                        pb = ps2.tile([128, 512], mybir.dt.float32, tag="p2",
                                      name=f"p2b{t}_{cb}")
                        cbs = slice(cb * 128, (cb + 1) * 128)
                        if cb < nb8:
                            # fp8 DoubleRow: psum = h8.T @ w2_8 = y * 2^12
                            for fi in range(NFI):
                                lhsT = h8_sb[:, 2 * fi : 2 * fi + 2, cbs]
                                first, last = (fi == 0), (fi == NFI - 1)
                                nc.tensor.matmul(
                                    pa[:], lhsT, w28_ap(fi, 0),
                                    start=first, stop=last, perf_mode=DR,
                                )
                                nc.tensor.matmul(
                                    pb[:], lhsT, w28_ap(fi, 1),
                                    start=first, stop=last, perf_mode=DR,
                                )
                        else:
                            for f in range(KO2):
                                lhsT = hT_sb[:, f, cbs]
                                first, last = (f == 0), (f == KO2 - 1)
                                nc.tensor.matmul(
                                    pa[:], lhsT, w2_ap(f, 0),
                                    start=first, stop=last,
                                )
                                nc.tensor.matmul(
                                    pb[:], lhsT, w2_ap(f, 1),
                                    start=first, stop=last,
                                )
                        ya = ypool.tile([128, 512], mybir.dt.float32, tag="y",
                                        name=f"ya{t}_{cb}")
                        yb = ypool.tile([128, 512], mybir.dt.float32, tag="y",
                                        name=f"yb{t}_{cb}")
                        # split evictions across DVE and ACT (raw copies; the
                        # fp8-tier 2^-12 descale happens on the host)
                        nc.vector.tensor_copy(ya[:], pa[:])
                        nc.scalar.copy(yb[:], pb[:])
                        r0 = t * c_tile + cb * 128
                        # split y across both DMA queues (xg+weights leave
                        # plenty of SP-queue headroom) to halve drain time
                        nc.scalar.dma_start(y[r0 : r0 + 128, 0:512], ya[:])
                        nc.sync.dma_start(y[r0 : r0 + 128, 512:1024], yb[:])

            if n_iters == 1:
                body()
            else:
                with tc.For_i(0, n_iters, 1):
                    body()
    nc.compile()
    return nc


def _get_nc():
    global _NC
    if _NC is None:
        _NC = _build_nc()
    return _NC


def _route(x, wr):
    """Replicates the reference router exactly (fp32 numpy), then permutes
    each expert's kept slots into ascending-ew order (low-ew slots first =
    the fp8 mm2 tier)."""
    xt = np.transpose(x, (1, 0, 2)).reshape(T, D)  # [T, D] fp32
    logits = xt.astype(np.float32) @ wr.astype(np.float32)  # [T, E]
    m = logits.max(axis=-1, keepdims=True)
    p = np.exp(logits - m, dtype=np.float32)
    p /= p.sum(axis=-1, keepdims=True)
    top1 = np.argmax(p, axis=-1)
    p_masked = p.copy()
    p_masked[np.arange(T), top1] = -np.inf
    top2 = np.argmax(p_masked, axis=-1)
    eidx = np.stack([top1, top2], axis=1)  # [T, K]
    ew = np.take_along_axis(p, eidx, axis=1).astype(np.float32)  # [T, K]

    flat_e = eidx.reshape(-1)
    order = np.argsort(flat_e, kind="stable")
    sorted_e = flat_e[order]
    hist = np.bincount(flat_e, minlength=E)
    starts = np.cumsum(hist) - hist
    pos = np.arange(T * K) - starts[sorted_e]
    keep = pos < C
    slot = np.where(keep, sorted_e * C + pos, E * C)
    token = order // K

    # permute kept slots within each expert by ascending ew (padding slots
    # have ew=0 and sort first, landing harmlessly in the fp8 tier)
    w_disp = ew.reshape(-1)[order]
    ew_slot = np.zeros(E * C, np.float32)
    ew_slot[slot[keep]] = w_disp[keep]
    slot_perm = np.empty(E * C + 1, np.int64)
    slot_perm[E * C] = E * C
    for e in range(E):
        seg = ew_slot[e * C : (e + 1) * C]
        rank = np.argsort(seg, kind="stable")
        newpos = np.empty(C, np.int64)
        newpos[rank] = np.arange(C)
        slot_perm[e * C : (e + 1) * C] = e * C + newpos
    slot = slot_perm[slot]
    return xt, ew, order, keep, slot, token


def _make_in_maps(x, wr, w1, w2):
    xt, ew, order, keep, slot, token = _route(x, wr)
    slot_token = np.zeros(E * C, np.int64)
    slot_token[slot[keep]] = token[keep]
    xT_bf = np.ascontiguousarray(xt.T.astype(BF16))  # [D, T]
    in_maps = []
    for e in range(E):
        idx = slot_token[e * C : (e + 1) * C]
        w2e = w2[e]
        w2_8 = np.clip(w2e * SW2, -240.0, 240.0).astype(F8)
        in_maps.append(
            {
                "xgT": np.ascontiguousarray(xT_bf[:, idx]),
                "w1": np.ascontiguousarray(w1[e].astype(BF16)),
                "w2": np.ascontiguousarray(w2e.astype(BF16)),
                "w2_8": np.ascontiguousarray(w2_8),
            }
        )
    return in_maps, (ew, order, keep, slot)


def kernel(x, wr, w1, w2):
    nc = _get_nc()
    in_maps, (ew, order, keep, slot) = _make_in_maps(x, wr, w1, w2)

    def _run_device():
        res = run_bass_kernel_spmd(nc, in_maps, core_ids=list(range(N_CORES)))
        # force materialization here so device errors surface inside the retry
        ys = [np.asarray(res.results[e]["y"], np.float32) for e in range(E)]
        if not all(np.isfinite(ye).all() for ye in ys):
            # a wedged core can return garbage without raising
            raise RuntimeError("non-finite device output")
        return ys

    Ys = None
    for attempt in range(3):
        try:
            Ys = _run_device()
            break
        except Exception:
            # transient device errors (wedged core) usually clear on retry
            if attempt == 2:
                raise

    # --- combine: weighted scatter back to tokens ---
    Y = np.empty((E * C, D), np.float32)
    nf8 = 128 * QF8
    for e in range(E):
        Y[e * C : (e + 1) * C] = Ys[e]
        # fp8-tier blocks hold y * 2^12 (raw psum); descale here
        Y[e * C : e * C + nf8] *= P2_INV

    inv = np.empty(T * K, np.int64)
    inv[order] = np.arange(T * K)
    slot_tk = slot[inv].reshape(T, K)
    keep_tk = keep[inv].reshape(T, K)

    out_flat = np.zeros((T, D), np.float32)
    for k in range(K):
        sl = np.clip(slot_tk[:, k], 0, E * C - 1)
        contrib = Y[sl] * ew[:, k : k + 1]
        contrib[~keep_tk[:, k]] = 0.0
        out_flat += contrib
    return np.ascontiguousarray(
        out_flat.reshape(S, B, D).transpose(1, 0, 2)
    ).astype(np.float32)


# ---------------------------------------------------------------------------
# Benchmark helper (used by test.py; not part of the grading contract).
# ---------------------------------------------------------------------------


def make_bench(in_maps):
    import jax
    from jax.experimental.shard_map import shard_map
    from jax.sharding import Mesh, PartitionSpec, NamedSharding
    from concourse.bass2jax import (
        _bass_exec_p,
        install_neuronx_cc_hook,
        partition_id_tensor,
    )

    nc = _NC if _NC is not None else _get_nc()
    install_neuronx_cc_hook()
    partition_name = nc.partition_id_tensor.name if nc.partition_id_tensor else None

    in_names, out_names, out_avals, zero_outs = [], [], [], []
    for alloc in nc.m.functions[0].allocations:
        if not isinstance(alloc, mybir.MemoryLocationSet):
            continue
        name = alloc.memorylocations[0].name
        if alloc.kind == "ExternalInput":
            if name != partition_name:
                in_names.append(name)
        elif alloc.kind == "ExternalOutput":
            shape = tuple(alloc.tensor_shape)
            dtype = mybir.dt.np(alloc.dtype)
            out_avals.append(jax.core.ShapedArray(shape, dtype))
            zero_outs.append(np.zeros(shape, dtype))
            out_names.append(name)
    n_params = len(in_names)
    all_in_names = list(in_names) + list(out_names)
    if partition_name is not None:
        all_in_names.append(partition_name)
    if nc.dbg_addr is not None:
        dbg_zero = np.zeros((1, 2), np.uint32)
        in_maps = [{**m, nc.dbg_addr.name: dbg_zero} for m in in_maps]

    def _body(*args):
        operands = list(args)
        if partition_name is not None:
            operands.append(partition_id_tensor())
        outs = _bass_exec_p.bind(
            *operands,
            out_avals=tuple(out_avals),
            in_names=tuple(all_in_names),
            out_names=tuple(out_names),
            lowering_input_output_aliases=(),
            sim_require_finite=True,
            sim_require_nnan=True,
            nc=nc,
        )
        return tuple(outs)

    devices = jax.devices()[:N_CORES]
    mesh = Mesh(np.asarray(devices), ("core",))
    n_outs = len(out_names)
    in_specs = (PartitionSpec("core"),) * (n_params + n_outs)
    out_specs = (PartitionSpec("core"),) * n_outs
    fn = jax.jit(
        shard_map(_body, mesh=mesh, in_specs=in_specs, out_specs=out_specs,
                  check_rep=False),
        keep_unused=True,
    )
    concat_in = [
        np.concatenate([np.asarray(in_maps[c][name]) for c in range(N_CORES)],
                       axis=0)
        for name in in_names
    ]
    concat_zeros = [
        np.zeros((N_CORES * z.shape[0], *z.shape[1:]), z.dtype)
        for z in zero_outs
    ]
    shard = NamedSharding(mesh, PartitionSpec("core"))
    args = [jax.device_put(a, shard) for a in concat_in + concat_zeros]
    return fn, args, out_names


def benchmark(in_maps, iters=20, warmup=3):
    import time
    import jax

    fn, args, out_names = make_bench(in_maps)
    for _ in range(warmup):
        out = fn(*args)
        jax.block_until_ready(out)
    times = []
    for _ in range(iters):
        t0 = time.perf_counter()
        out = fn(*args)
        jax.block_until_ready(out)
        times.append(time.perf_counter() - t0)
    return min(times), sorted(times)[len(times) // 2], out
